# revision 27
# baseline (speedup 1.0000x reference)
"""Trainium2 Bass kernel for nn_DWTenhance (Haar DWT + dual MDTA + inverse DWT).

Exact algorithmic reformulation (same as baseline):
  Per sample the whole network is an affine map of the 2x2-polyphase
  representation P (256 x n, n = 128*128) of the image:
      out_polyphase = W_big @ P + b_big,
  where W_big depends on the data only through the 256x256 Gram matrix
  C = P@P^T and the row sums s = P@1.

Performance layout (vs baseline's two-f32-HBM-pass design):
  * Host converts the image to bf16 and pre-splits the polyphase parity:
    DRAM img[s, p=(dy,ci), h2, dx, w2].  HBM read traffic is halved and
    every DMA run is 8 KiB contiguous per partition.
  * The whole per-core input (2 samples x 8 MB bf16) stays SBUF-resident:
    pass 1 (Gram) and pass 2 (apply W) read SBUF, so HBM is touched once.
  * All large matmuls/transposes run in bf16 (1 PE cycle/row; f32
    transposes cost 2).  Output is written bf16 and upcast on host.
  * Gram uses symmetry: only C[0:128,:] and C[128:,128:] are accumulated;
    the missing block is one PE transpose of the f32 result.
  * tiny() (per-sample small-matrix algebra + softmax) is emitted
    interleaved with the next sample's Gram / previous sample's apply so
    its serial PE<->DVE chain hides under streaming work.

Data parallel over batch: 16 samples / 8 NeuronCores = 2 samples per core.
"""
import sys

sys.path.insert(0, '/opt/trn_rl_repo')

import numpy as np
import ml_dtypes

import concourse.bass as bass
import concourse.tile as tile
from concourse import bacc, mybir
from concourse.masks import make_identity

F32 = mybir.dt.float32
F32R = mybir.dt.float32r
BF16 = mybir.dt.bfloat16
FP8 = mybir.dt.float8e4
DROW = mybir.MatmulPerfMode.DoubleRow
AX = mybir.AxisListType
ALU = mybir.AluOpType
ACTF = mybir.ActivationFunctionType

B, C, H, W = 16, 64, 256, 256
NCORES = 8
SPC = B // NCORES           # samples per core
H2, W2 = H // 2, W // 2
NPIX = H2 * W2              # 16384
PC = 4 * C                  # 256 polyphase channels
CH = 16                     # half-res rows per streamed chunk
NCHK = H2 // CH             # chunks per sample
EPS = 1e-12

BRANCHES = (('l', C), ('h', 3 * C))


def _pidx(dx, dy, ci):
    # polyphase channel order; must match the pass-1 transpose layout
    return dx * 2 * C + dy * C + ci


def _build_AN_SY():
    AN = np.zeros((PC, PC))
    SY = np.zeros((PC, PC))
    for ci in range(C):
        a, b, c_, d = (_pidx(0, 0, ci), _pidx(1, 0, ci),
                       _pidx(0, 1, ci), _pidx(1, 1, ci))
        AN[ci, [a, b, c_, d]] = [0.5, 0.5, 0.5, 0.5]
        AN[C + 3 * ci + 0, [a, b, c_, d]] = [0.5, -0.5, 0.5, -0.5]
        AN[C + 3 * ci + 1, [a, b, c_, d]] = [0.5, 0.5, -0.5, -0.5]
        AN[C + 3 * ci + 2, [a, b, c_, d]] = [0.5, -0.5, -0.5, 0.5]
        l, h1, h2, h3 = ci, C + 3 * ci, C + 3 * ci + 1, C + 3 * ci + 2
        SY[a, [l, h1, h2, h3]] = [0.5, 0.5, 0.5, 0.5]
        SY[b, [l, h1, h2, h3]] = [0.5, -0.5, 0.5, -0.5]
        SY[c_, [l, h1, h2, h3]] = [0.5, 0.5, -0.5, -0.5]
        SY[d, [l, h1, h2, h3]] = [0.5, -0.5, -0.5, 0.5]
    return AN, SY


def _chunks(c):
    out = []
    i = 0
    while i < c:
        out.append((i, min(128, c - i)))
        i += 128
    return out


def build_host_consts(inputs):
    """float64 host preprocessing of the small weights -> DRAM const arrays."""
    AN, SY = _build_AN_SY()
    f = np.float64
    consts = {}
    temps = {}
    bias_vec = np.zeros(PC)
    for br, key, c in (('l', 'll', C), ('h', 'h', 3 * C)):
        qw, qb = inputs[f'{key}_qw'].astype(f), inputs[f'{key}_qb'].astype(f)
        kw, kb = inputs[f'{key}_kw'].astype(f), inputs[f'{key}_kb'].astype(f)
        vw, vb = inputs[f'{key}_vw'].astype(f), inputs[f'{key}_vb'].astype(f)
        pw, pb = inputs[f'{key}_pw'].astype(f), inputs[f'{key}_pb'].astype(f)
        temps[br] = float(np.asarray(inputs[f'{key}_temp']).reshape(-1)[0])
        ANb = AN[:C] if br == 'l' else AN[C:]
        SYb = SY[:, :C] if br == 'l' else SY[:, C:]
        Wq = qw @ ANb          # (c, 256)
        Wk = kw @ ANb
        VAN = vw @ ANb         # (c, 256)
        PS = SYb @ pw          # (256, c)
        chs = _chunks(c)
        mc = len(chs)
        consts[f'wqt_{br}'] = np.ascontiguousarray(
            Wq.T.reshape(2, 128, c)).astype(np.float32)
        consts[f'wkt_{br}'] = np.ascontiguousarray(
            Wk.T.reshape(2, 128, c)).astype(np.float32)
        van = np.zeros((mc, 128, PC))
        pst = np.zeros((mc, 128, PC))
        eye = np.zeros((mc, 128, c))
        vbf = np.zeros((mc, 128, c))
        for mi, (o, sz) in enumerate(chs):
            van[mi, :sz] = VAN[o:o + sz]
            pst[mi, :sz] = PS.T[o:o + sz]
            eye[mi, :sz, o:o + sz] = np.eye(sz)
            vbf[mi, :, :] = vb[None, :]
        consts[f'van_{br}'] = van.astype(np.float32)
        consts[f'pst_{br}'] = pst.astype(np.float32)
        consts[f'eye_{br}'] = eye.astype(np.float32)
        consts[f'vbf_{br}'] = vbf.astype(np.float32)

        def colchunks(v):
            arr = np.zeros((mc, 128, 1))
            for mi, (o, sz) in enumerate(chs):
                arr[mi, :sz, 0] = v[o:o + sz]
            return arr.astype(np.float32)
        consts[f'qb2_{br}'] = colchunks(2.0 * qb)
        consts[f'qb2n_{br}'] = colchunks(NPIX * qb * qb)
        consts[f'kb2_{br}'] = colchunks(2.0 * kb)
        consts[f'kb2n_{br}'] = colchunks(NPIX * kb * kb)
        consts[f'qbrow_{br}'] = qb[None, :].astype(np.float32)
        consts[f'kbrow_{br}'] = kb[None, :].astype(np.float32)
        bias_vec += SYb @ pb
    consts['bconst'] = np.ascontiguousarray(
        bias_vec.reshape(2, 128, 1)).astype(np.float32)
    return consts, temps


CSHAPES = {
    'wqt_l': [2, 128, C], 'wkt_l': [2, 128, C],
    'wqt_h': [2, 128, 3 * C], 'wkt_h': [2, 128, 3 * C],
    'van_l': [1, 128, PC], 'pst_l': [1, 128, PC],
    'van_h': [2, 128, PC], 'pst_h': [2, 128, PC],
    'eye_l': [1, 128, C], 'vbf_l': [1, 128, C],
    'eye_h': [2, 128, 3 * C], 'vbf_h': [2, 128, 3 * C],
    'qb2_l': [1, 128, 1], 'qb2n_l': [1, 128, 1],
    'kb2_l': [1, 128, 1], 'kb2n_l': [1, 128, 1],
    'qb2_h': [2, 128, 1], 'qb2n_h': [2, 128, 1],
    'kb2_h': [2, 128, 1], 'kb2n_h': [2, 128, 1],
    'qbrow_l': [1, C], 'kbrow_l': [1, C],
    'qbrow_h': [1, 3 * C], 'kbrow_h': [1, 3 * C],
    'bconst': [2, 128, 1],
}


DEFAULT_CFG = dict(pt_bufs=2, osb_bufs=3, tpp_bufs=2, op_bufs=3,
                   tps_bufs=1, cps_bufs=1, work_bufs=1, w_bufs=2,
                   steps=2, loops=1)


def build_program(temps, cfg=None):
    """Build the Bacc program one core runs (SPC samples, full pipeline)."""
    cfg = dict(DEFAULT_CFG, **(cfg or {}))
    nc = bacc.Bacc()

    imgd = nc.declare_dram_parameter('img', [SPC, 128, H2, 2, W2], BF16,
                                     isOutput=False)
    outd = nc.declare_dram_parameter('out', [SPC, 128, H2, 2, W2], BF16,
                                     isOutput=True)
    cdecl = {n: nc.declare_dram_parameter(n, s, F32, isOutput=False)
             for n, s in CSHAPES.items()}

    with tile.TileContext(nc) as tc:
        with tc.tile_pool(name='cst', bufs=1) as cst, \
             tc.tile_pool(name='ptp', bufs=cfg['pt_bufs']) as ptp, \
             tc.tile_pool(name='io', bufs=cfg['osb_bufs']) as io, \
             tc.tile_pool(name='work', bufs=cfg['work_bufs']) as work, \
             tc.tile_pool(name='wp', bufs=cfg['w_bufs']) as wpool, \
             tc.tile_pool(name='xp', bufs=cfg['tpp_bufs'], space='PSUM') as xp, \
             tc.tile_pool(name='xp2', bufs=cfg['op_bufs'], space='PSUM') as xp2, \
             tc.tile_pool(name='cps', bufs=cfg['cps_bufs'], space='PSUM') as cps, \
             tc.tile_pool(name='tps', bufs=1, space='PSUM') as tps:

            # ---------------- constants ----------------
            identf = cst.tile([128, 128], F32)
            make_identity(nc, identf)
            identb = cst.tile([128, 128], BF16)
            make_identity(nc, identb)
            ones2b = cst.tile([128, 2], FP8)
            nc.vector.memset(ones2b, 1.0)
            ones_row = cst.tile([1, 128], F32)
            nc.vector.memset(ones_row, 1.0)
            zcst = cst.tile([128, PC], F32)
            nc.gpsimd.memset(zcst, 0.0)
            csb = {}

            def load_consts():
                # emitted AFTER sample 0's img DMAs so the PE can start
                # transposing ~3us in instead of waiting out 25 const DMAs
                for name, shp in CSHAPES.items():
                    dt = F32
                    if len(shp) == 3:
                        t = cst.tile([128, shp[0], shp[2]], dt, tag=name,
                                     name=name)
                        nc.sync.dma_start(
                            out=t,
                            in_=cdecl[name][:, :, :].rearrange('c p x -> p c x'))
                    else:
                        t = cst.tile([1, shp[1]], dt, tag=name, name=name)
                        nc.sync.dma_start(out=t, in_=cdecl[name][:, :])
                    csb[name] = t

            CFG_TPS = cfg['tps_bufs']

            def cget(name):
                return csb[name]

            # SBUF-resident image chunks: [128=(dy,ci), CH rows, dx, W2]
            imgt = [[cst.tile([128, CH, 2, W2], BF16, tag=f'img{s}_{k}',
                              name=f'img{s}_{k}')
                     for k in range(NCHK)] for s in range(SPC)]

            def dma_in(s):
                for k in range(NCHK):
                    nc.sync.dma_start(
                        out=imgt[s][k],
                        in_=imgd[s, :, k * CH:(k + 1) * CH, :, :])

            # =============== PASS 1: Gram accumulation ===============
            def pass1_gen(s, cp):
                cp0, cp1 = cp
                for k in range(NCHK):
                    src = imgt[s][k]
                    for half in range(2):
                        pt = ptp.tile([128, CH // 2, 384], FP8, tag='pt',
                                      name='pt')
                        nc.vector.tensor_copy(
                            out=pt[:, :, 256:258],
                            in_=ones2b[:, None, :].to_broadcast(
                                (128, CH // 2, 2)))
                        for grp in range(2):
                            tpp = xp.tile([128, 4, 2, 128], BF16, tag='tpp',
                                          name='tpp')
                            for tt in range(4):
                                t = half * (CH // 2) + grp * 4 + tt
                                for dx in range(2):
                                    nc.tensor.transpose(
                                        tpp[:, tt, dx], src[:, t, dx, :],
                                        identb[:])
                            dst = pt[:, grp * 4:grp * 4 + 4, 0:256]
                            if (half + grp) % 2 == 0:
                                nc.vector.tensor_copy(out=dst, in_=tpp[:])
                            else:
                                nc.scalar.activation(
                                    out=dst, in_=tpp[:],
                                    func=ACTF.Identity, bias=0.0, scale=1.0)
                        for tt in range(0, CH // 2, 2):
                            t = k * CH + half * (CH // 2) + tt
                            first = (t == 0)
                            last = (t == H2 - 2)
                            nc.tensor.matmul(
                                cp0[:, 0:258], pt[:, tt:tt + 2, 0:128],
                                pt[:, tt:tt + 2, 0:258], start=first,
                                stop=last, perf_mode=DROW)
                            nc.tensor.matmul(
                                cp1[:, 0:130], pt[:, tt:tt + 2, 128:256],
                                pt[:, tt:tt + 2, 128:258], start=first,
                                stop=last, perf_mode=DROW)
                    yield k

            def make_csb(s, cp):
                cp0, cp1 = cp
                c_sb = work.tile([128, 2, 258], F32, tag='csb', name='csb')
                nc.vector.tensor_copy(out=c_sb[:, 0, :], in_=cp0[:])
                nc.scalar.activation(out=c_sb[:, 1, 128:258], in_=cp1[:],
                                     func=ACTF.Identity, bias=0.0, scale=1.0)
                tr = tps.tile([128, 258], F32, tag='tps2', name='tr')
                nc.tensor.transpose(tr[:, 0:128], c_sb[:, 0, 128:256],
                                    identf[:])
                nc.vector.tensor_copy(out=c_sb[:, 1, 0:128], in_=tr[:, 0:128])
                return c_sb

            # =============== tiny per-sample math ===============
            def tiny_gen(s, c_sb, res):
                a_sb = {}
                for br, c in BRANCHES:
                    chs = _chunks(c)
                    mc = len(chs)
                    wqt, wkt = cget(f'wqt_{br}'), cget(f'wkt_{br}')
                    vq_sb = work.tile([128, 2, c], F32, tag=f'vq_{br}',
                                      name='vq')
                    vk_sb = work.tile([128, 2, c], F32, tag=f'vk_{br}',
                                      name='vk')
                    for (vsb, wt) in ((vq_sb, wqt), (vk_sb, wkt)):
                        for rc in range(2):
                            vp = tps.tile([128, 258], F32, tag='tps',
                                          bufs=CFG_TPS, name='vp')
                            for mi in range(2):
                                nc.tensor.matmul(
                                    vp[:, 0:c],
                                    c_sb[:, mi, rc * 128:(rc + 1) * 128],
                                    wt[:, mi, :],
                                    start=(mi == 0), stop=(mi == 1))
                            nc.vector.tensor_copy(out=vsb[:, rc, :],
                                                  in_=vp[:, 0:c])
                    yield
                    dcol = {}
                    for (dtag, wt, vsb) in (('q', wqt, vq_sb),
                                            ('k', wkt, vk_sb)):
                        dc = work.tile([128, mc, 1], F32, tag=f'd{dtag}_{br}',
                                       name='dc')
                        for mi, (o, sz) in enumerate(chs):
                            gp = tps.tile([128, 258], F32, tag='tps',
                                          bufs=CFG_TPS, name='gqp')
                            for rc in range(2):
                                nc.tensor.matmul(
                                    gp[:sz, 0:c],
                                    wt[:, rc, o:o + sz],
                                    vsb[:, rc, :],
                                    start=(rc == 0), stop=(rc == 1))
                            tmp = work.tile([128, c], F32, tag=f'tmp_{br}',
                                            name='dtmp')
                            nc.vector.tensor_tensor(
                                out=tmp[:sz, :], in0=gp[:sz, 0:c],
                                in1=cget(f'eye_{br}')[:sz, mi, :],
                                op=ALU.mult)
                            nc.vector.tensor_reduce(
                                out=dc[:sz, mi, :], in_=tmp[:sz, :],
                                axis=AX.X, op=ALU.add)
                        dcol[dtag] = dc
                    yield
                    rows = {}
                    for (dtag, wt) in (('q', wqt), ('k', wkt)):
                        rp = tps.tile([128, 258], F32, tag='tps',
                                      bufs=CFG_TPS, name='rp')
                        for rc in range(2):
                            nc.tensor.matmul(
                                rp[0:1, 0:c], c_sb[:, rc, 256:257],
                                wt[:, rc, :], start=(rc == 0), stop=(rc == 1))
                        rsb = work.tile([1, c], F32, tag=f'{dtag}row_{br}',
                                        name='rsb')
                        nc.vector.tensor_copy(out=rsb[:], in_=rp[0:1, 0:c])
                        rows[dtag] = rsb
                    cols = {}
                    for (dtag, wt) in (('q', wqt), ('k', wkt)):
                        ccol = work.tile([128, mc, 1], F32,
                                         tag=f'{dtag}col_{br}', name='ccol')
                        for mi, (o, sz) in enumerate(chs):
                            cp_ = tps.tile([128, 8], F32, tag='tps2',
                                           name='cp')
                            for rc in range(2):
                                nc.tensor.matmul(
                                    cp_[:sz, 0:1], wt[:, rc, o:o + sz],
                                    c_sb[:, rc, 256:257],
                                    start=(rc == 0), stop=(rc == 1))
                            nc.vector.tensor_copy(out=ccol[:sz, mi, :],
                                                  in_=cp_[:sz, 0:1])
                        cols[dtag] = ccol
                    yield
                    rq_sb = work.tile([128, mc, 1], F32, tag=f'rq_{br}',
                                      name='rq')
                    sk_sb = work.tile([128, mc, 1], F32, tag=f'sk_{br}',
                                      name='skc')
                    for mi, (o, sz) in enumerate(chs):
                        nc.vector.tensor_tensor(
                            out=rq_sb[:sz, mi, :], in0=cols['q'][:sz, mi, :],
                            in1=cget(f'qb2_{br}')[:sz, mi, :], op=ALU.mult)
                        nc.vector.tensor_add(
                            out=rq_sb[:sz, mi, :], in0=rq_sb[:sz, mi, :],
                            in1=dcol['q'][:sz, mi, :])
                        nc.vector.tensor_add(
                            out=rq_sb[:sz, mi, :], in0=rq_sb[:sz, mi, :],
                            in1=cget(f'qb2n_{br}')[:sz, mi, :])
                        nc.scalar.activation(
                            out=rq_sb[:sz, mi, :], in_=rq_sb[:sz, mi, :],
                            func=ACTF.Sqrt, bias=0.0, scale=1.0)
                        nc.vector.tensor_scalar_max(
                            rq_sb[:sz, mi, :], rq_sb[:sz, mi, :], EPS)
                        nc.vector.reciprocal(out=rq_sb[:sz, mi, :],
                                             in_=rq_sb[:sz, mi, :])
                        nc.vector.tensor_scalar_mul(
                            rq_sb[:sz, mi, :], rq_sb[:sz, mi, :],
                            float(temps[br]))
                        nc.vector.tensor_tensor(
                            out=sk_sb[:sz, mi, :], in0=cols['k'][:sz, mi, :],
                            in1=cget(f'kb2_{br}')[:sz, mi, :], op=ALU.mult)
                        nc.vector.tensor_add(
                            out=sk_sb[:sz, mi, :], in0=sk_sb[:sz, mi, :],
                            in1=dcol['k'][:sz, mi, :])
                        nc.vector.tensor_add(
                            out=sk_sb[:sz, mi, :], in0=sk_sb[:sz, mi, :],
                            in1=cget(f'kb2n_{br}')[:sz, mi, :])
                    skr = tps.tile([128, 258], F32, tag='tps', bufs=CFG_TPS,
                                   name='skr')
                    for mi, (o, sz) in enumerate(chs):
                        nc.tensor.matmul(
                            skr[0:1, 0:c], sk_sb[:sz, mi, :],
                            cget(f'eye_{br}')[:sz, mi, :],
                            start=(mi == 0), stop=(mi == mc - 1))
                    rk_sb = work.tile([1, c], F32, tag=f'rk_{br}', name='rk')
                    nc.scalar.activation(out=rk_sb[:], in_=skr[0:1, 0:c],
                                         func=ACTF.Sqrt, bias=0.0, scale=1.0)
                    nc.vector.tensor_scalar_max(rk_sb[:], rk_sb[:], EPS)
                    nc.vector.reciprocal(out=rk_sb[:], in_=rk_sb[:])
                    tmr = work.tile([1, c], F32, tag=f'tmr_{br}', name='tmr')
                    nc.vector.tensor_scalar_mul(
                        tmr[:], cget(f'kbrow_{br}')[:], float(NPIX))
                    nc.vector.tensor_add(out=tmr[:], in0=tmr[:],
                                         in1=rows['k'][:])
                    a_t = work.tile([128, mc, c], F32, tag=f'a_{br}',
                                    name='a_t')
                    zmi = 1 if br == 'h' else 0
                    nc.vector.tensor_copy(out=a_t[64:128, zmi, :],
                                          in_=zcst[0:64, 0:c])
                    yield
                    for mi, (o, sz) in enumerate(chs):
                        gp = tps.tile([128, 258], F32, tag='tps',
                                      bufs=CFG_TPS, name='gp')
                        for rc in range(2):
                            nc.tensor.matmul(
                                gp[:sz, 0:c], wqt[:, rc, o:o + sz],
                                vk_sb[:, rc, :], start=(rc == 0), stop=False)
                        nc.tensor.matmul(
                            gp[:sz, 0:c], rows['q'][:, o:o + sz],
                            cget(f'kbrow_{br}')[:], start=False, stop=False)
                        nc.tensor.matmul(
                            gp[:sz, 0:c], cget(f'qbrow_{br}')[:, o:o + sz],
                            tmr[:], start=False, stop=True)
                        rkp = tps.tile([128, 258], F32, tag='tps2', name='rkp')
                        nc.tensor.matmul(rkp[:sz, 0:c], ones_row[:, 0:sz],
                                         rk_sb[:], start=True, stop=True)
                        s_t = work.tile([128, c], F32, tag=f's_{br}',
                                        name='s_t')
                        nc.vector.tensor_scalar_mul(
                            s_t[:sz, :], gp[:sz, 0:c], rq_sb[:sz, mi, :])
                        nc.vector.tensor_tensor(
                            out=s_t[:sz, :], in0=s_t[:sz, :],
                            in1=rkp[:sz, 0:c], op=ALU.mult)
                        nmax = work.tile([128, 1], F32, tag=f'nm_{br}',
                                         name='nmax')
                        nc.vector.tensor_reduce(
                            out=nmax[:sz], in_=s_t[:sz, :], axis=AX.X,
                            op=ALU.max, negate=True)
                        nc.scalar.activation(
                            out=a_t[:sz, mi, :], in_=s_t[:sz, :],
                            func=ACTF.Exp, bias=nmax[:sz], scale=1.0)
                        ssum = work.tile([128, 1], F32, tag=f'ss_{br}',
                                         name='ssum')
                        nc.vector.tensor_reduce(
                            out=ssum[:sz], in_=a_t[:sz, mi, :], axis=AX.X,
                            op=ALU.add)
                        nc.vector.reciprocal(out=ssum[:sz], in_=ssum[:sz])
                        nc.vector.tensor_scalar_mul(
                            a_t[:sz, mi, :], a_t[:sz, mi, :], ssum[:sz])
                        yield
                    a_sb[br] = a_t

                t2 = {}
                for br, c in BRANCHES:
                    chs = _chunks(c)
                    mc = len(chs)
                    t2_sb = work.tile([128, mc, PC], F32, tag=f't2_{br}',
                                      name='t2sb')
                    for ji, (jo, jsz) in enumerate(chs):
                        if jsz < 128:
                            nc.vector.tensor_copy(
                                out=t2_sb[jsz:128, ji, :],
                                in_=zcst[0:128 - jsz, 0:PC])
                        tp2 = tps.tile([128, 258], F32, tag='tps',
                                       bufs=CFG_TPS, name='tp2')
                        for mi in range(mc):
                            nc.tensor.matmul(
                                tp2[:jsz, 0:PC],
                                a_sb[br][:, mi, jo:jo + jsz],
                                cget(f'pst_{br}')[:, mi, :],
                                start=(mi == 0), stop=(mi == mc - 1))
                        nc.vector.tensor_copy(out=t2_sb[:jsz, ji, :],
                                              in_=tp2[:jsz, 0:PC])
                        yield
                    t2[br] = t2_sb

                w_t = wpool.tile([128, 2, PC], BF16, tag='wsb', name='wsb')
                seq = [(br, ji) for br, c in BRANCHES
                       for ji in range(len(_chunks(c)))]
                for nch in range(2):
                    wp = tps.tile([128, 258], F32, tag='tps', bufs=CFG_TPS,
                                  name='wp')
                    for i, (br, ji) in enumerate(seq):
                        nc.tensor.matmul(
                            wp[:, 0:PC],
                            cget(f'van_{br}')[:, ji, nch * 128:(nch + 1) * 128],
                            t2[br][:, ji, :],
                            start=(i == 0), stop=(i == len(seq) - 1))
                    nc.vector.tensor_copy(out=w_t[:, nch, :], in_=wp[:, 0:PC])
                    yield

                pvs = {}
                for br, c in BRANCHES:
                    chs = _chunks(c)
                    pv = work.tile([128, len(chs), 1], F32, tag=f'pv_{br}',
                                   name='pv')
                    for mi, (o, sz) in enumerate(chs):
                        tmp = work.tile([128, c], F32, tag=f'tmp_{br}',
                                        name='pvt')
                        nc.vector.tensor_tensor(
                            out=tmp[:, :], in0=a_sb[br][:, mi, :],
                            in1=cget(f'vbf_{br}')[:, mi, :], op=ALU.mult)
                        with nc.allow_low_precision(
                                reason='f32r is f32 bitwise'):
                            nc.vector.tensor_reduce(
                                out=pv[:, mi, :], in_=tmp[:, :], axis=AX.X,
                                op=ALU.add)
                    pvs[br] = pv
                b_t = work.tile([128, 2, 1], F32, tag='bcol', name='bcol')
                seqb = [(br, mi) for br, c in BRANCHES
                        for mi in range(len(_chunks(c)))]
                for nch in range(2):
                    bp = tps.tile([128, 8], F32, tag='tps2', name='bp')
                    for i, (br, mi) in enumerate(seqb):
                        nc.tensor.matmul(
                            bp[:, 0:1],
                            cget(f'pst_{br}')[:, mi, nch * 128:(nch + 1) * 128],
                            pvs[br][:, mi, :],
                            start=(i == 0), stop=(i == len(seqb) - 1))
                    nc.vector.tensor_tensor(
                        out=b_t[:, nch, :], in0=bp[:, 0:1],
                        in1=cget('bconst')[:, nch, :], op=ALU.add)
                res['w'] = w_t
                res['b'] = b_t

            # =============== PASS 2: apply W_big^T ===============
            def pass2_gen(s, res):
                w_t, b_t = res['w'], res['b']
                for k in range(NCHK):
                    src = imgt[s][k]
                    osb = io.tile([128, CH, 2, W2], BF16, tag='osb',
                                  name='osb')
                    for sub in range(CH // 4):
                        rows = slice(sub * 4, sub * 4 + 4)
                        for nch in range(2):
                            op = xp2.tile([128, 4, 128], F32, tag='op',
                                          name='op')
                            for dxin in range(2):
                                nc.tensor.matmul(
                                    op[:, :, :],
                                    w_t[:, dxin, nch * 128:(nch + 1) * 128],
                                    src[:, rows, dxin, :],
                                    start=(dxin == 0), stop=(dxin == 1))
                            if (sub + nch) % 2 == 0:
                                nc.scalar.activation(
                                    out=osb[:, rows, nch, :], in_=op[:],
                                    func=ACTF.Identity,
                                    bias=b_t[:, nch, :], scale=1.0)
                            else:
                                nc.vector.tensor_scalar_add(
                                    osb[:, rows, nch, :], op[:],
                                    b_t[:, nch, :])
                    nc.sync.dma_start(
                        out=outd[s, :, k * CH:(k + 1) * CH, :, :],
                        in_=osb)
                    yield k

            # =============== schedule ===============
            def interleave(chunk_gen, tiny_it, steps):
                for _ in chunk_gen:
                    if tiny_it is not None:
                        for _ in range(steps):
                            if next(tiny_it, StopIteration) is StopIteration:
                                tiny_it = None
                                break
                if tiny_it is not None:
                    for _ in tiny_it:
                        pass

            def body(_i=None):
                steps = cfg['steps']
                dma_in(0)
                load_consts()
                dma_in(1)
                cpt0 = cps.tile([128, 388], F32, tag='cp', name='cp')
                cp0 = [cpt0[:, 0:258], cpt0[:, 258:388]]
                for _ in pass1_gen(0, cp0):
                    pass
                c_sb0 = make_csb(0, cp0)
                res0 = {}
                t0 = tiny_gen(0, c_sb0, res0)
                cpt1 = cps.tile([128, 388], F32, tag='cp', name='cp')
                cp1 = [cpt1[:, 0:258], cpt1[:, 258:388]]
                interleave(pass1_gen(1, cp1), t0, steps)
                c_sb1 = make_csb(1, cp1)
                res1 = {}
                t1 = tiny_gen(1, c_sb1, res1)
                interleave(pass2_gen(0, res0), t1, steps)
                for _ in pass2_gen(1, res1):
                    pass

            if cfg['loops'] == 1:
                body()
            else:
                with tc.For_i(0, cfg['loops'], 1) as _i:
                    body(_i)

    nc.finalize()
    return nc


_CACHE = {}


def get_program(temps, cfg=None):
    key = (tuple(sorted(temps.items())),
           tuple(sorted((cfg or {}).items(), key=str)))
    if key not in _CACHE:
        _CACHE[key] = build_program(temps, cfg)
    return _CACHE[key]


def host_pack_img(img):
    """f32 [B,C,H,W] -> bf16 [B, 128=(dy,ci), h2, dx, w2]."""
    x = img.reshape(-1, C, H2, 2, W2, 2).transpose(0, 3, 1, 2, 5, 4)
    return np.ascontiguousarray(x).reshape(-1, 2 * C // 2 * 2, H2, 2, W2) \
        .astype(ml_dtypes.bfloat16)


def host_unpack_out(outp):
    """bf16 [B, 128=(dy,ci), h2, dx, w2] -> f32 [B,C,H,W]."""
    x = outp.astype(np.float32).reshape(-1, 2, C, H2, 2, W2)
    x = x.transpose(0, 2, 3, 1, 5, 4)
    return np.ascontiguousarray(x).reshape(-1, C, H, W)


def make_in_maps(inputs):
    inputs = {k: np.asarray(v) for k, v in inputs.items()}
    consts, temps = build_host_consts(inputs)
    imgp = host_pack_img(np.ascontiguousarray(inputs['img'],
                                              dtype=np.float32))
    in_maps = []
    for core in range(NCORES):
        m = {'img': imgp[core * SPC:(core + 1) * SPC]}
        m.update(consts)
        in_maps.append(m)
    return in_maps, temps


def kernel(**inputs):
    in_maps, temps = make_in_maps(inputs)
    nc = get_program(temps)
    from concourse.bass_utils import run_bass_kernel_spmd
    res = run_bass_kernel_spmd(nc, in_maps, core_ids=list(range(NCORES)),
                               trace=False)
    outp = np.concatenate([np.asarray(res.results[c]['out'])
                           for c in range(NCORES)], axis=0)
    return host_unpack_out(outp).astype(np.float32)
